# revision 40
# baseline (speedup 1.0000x reference)
"""DGINConv (2-layer GIN with edge features) Trainium2 kernel — sparse/packed.

Math (per layer, reference):
    hb[j,:] = Wnode @ x[j] + bne                       # [N, D] node term
    he[i,j,:] = We @ edges[i,j,:]                      # edge term
    msg[i,:] = sum_{j: adj[i,j]=1} relu(hb[j,:] + he[i,j,:])
    out = relu(Wn @ ((1+eps)*x[i] + msg[i]) + bn)

adj density is ~3%, so instead of the dense [128 own-rows x 1024 j] sweep we
pack each own row's ~31 neighbors into padded slots (host-side, from the
actual adj at runtime):
  - own rows sorted by degree (host permutation), grouped into 8 chunks of
    16 rows; chunk c padded to S_c slots/row (mult of 4).  Q = 16*sum(S_c).
  - packed edge vectors -> peT [32, Q] bf16 (zero for pad slots)
  - slot -> source-node index list (int16), PAD slots point at column 1024
    of hbT which holds -1e9, so relu(hb_pad + 0) == 0.

Device per layer:
  hbT[d, j] = Wnode @ xT + bne (PE + ACT);  pads = -1e9
  hbg[d, q] = ap_gather(hbT, idx)           (POOL custom ucode op)
  psum[d, q] = We @ peT                     (PE, K=32)
  r[d, q] = relu(psum + hbg)                (DVE custom relu(a+b) op, or
                                             POOL add + ACT relu)
  msg[d, i] = segment-sum over S_c slots    (DVE bf16 2x tree + tensor_reduce)
  h = relu(Wn @ ((1+eps)x + msg) + bn)      (PE + ACT)

Between layers: transpose h1 -> [i,d], AllGather (rows stay in per-core
sorted order; layer-2 gather indices are host-remapped to that layout).
Final output rows are un-permuted on the host.

Distribution: destination rows sharded 8 ways; nodes/weights replicated;
one AllGather of updated node features between layers.
"""

import sys

if "/opt/trn_rl_repo" not in sys.path:
    sys.path.insert(0, "/opt/trn_rl_repo")

import numpy as np

N, D, E, NC = 1024, 128, 32, 8
SH = N // NC          # 128 rows per core
NCH = 8               # chunks of sorted own-rows
CHI = SH // NCH       # 16 rows per chunk
PAD = N               # hbT column holding -1e9
HBW = N + 8           # hbT width (pad cols 1024..1032)

# Chunk slot counts (padded max degree per 16-row chunk of the degree-sorted
# rows, mult of 4, same for all cores).  Recomputed from the actual adj at
# runtime; this is the value for the reference setup_inputs() graph.
S_DEFAULT = (52, 40, 36, 36, 32, 32, 28, 28)

# engine assignment knobs (tuned against TimelineSim)
L0_EXIT = "AAAAAAAA"   # 'A' ACT relu | 'D' DVE tensor_scalar relu
L1_EXIT = "PPPPDDDD"   # 'D' DVE custom relu(a+b) | 'A' ACT convert + PE inject
TREE0 = "DDDDDDDD"     # 'D' DVE | 'P' POOL (L1+L2 tree levels)
TREE1 = "DDDDDDDD"
GATHER_SPLIT = 4       # ap_gather instructions per layer
HB_SPLIT = False       # split hb-exit across DVE+ACT
HB_ENG = "D"           # hb-exit engine when not split: 'D' DVE | 'A' ACT
CONV_PREP = False      # emit l1 converts inside hb_prep (early pool/ACT order)
IDX_EARLY = False      # idx12 DMA before peTA pieces

_cache = {}
_CUSTOM = {}


def _ensure_custom_op():
    """Register RELU_ADD_REDUCE_GIN: out = relu(in0 + in1); accum = sum."""
    if "op" in _CUSTOM:
        return _CUSTOM["op"]
    import concourse.dve_ops as dve_ops
    from concourse.dve_spec import Spec, Src0, Src1, relu, lower, _has_src1
    from concourse.dve_spec import Zero
    from concourse.dve_uop import DveOpSpec
    from operator import add

    name = "RELU_ADD_REDUCE_GIN"

    def _ref(in0, in1, c0, c1, c2):
        b = dve_ops._dve_relu(in0.astype(np.float32) + in1.astype(np.float32))
        return b, b.reshape(b.shape[0], -1).sum(axis=-1, keepdims=True)

    spec = Spec(body=relu(Src0 + Src1), accum=add, accum_init=Zero,
                reference=_ref)
    row = dve_ops._CUSTOM_DVE_ROW_BASE + len(dve_ops.OPS)
    assert row < 0x20
    shas = {}
    for ver in ("v3", "v4"):
        try:
            s = DveOpSpec(name=name, opcode=row, uops=lower(spec, ver=ver),
                          rd1_en=_has_src1(spec))
            shas[ver] = s.sha(ver)
        except Exception:
            pass
    op = dve_ops.DveOp(name, spec, subdim=False, uops_sha=shas)
    dve_ops.OPS.append(op)
    dve_ops.CUSTOM_DVE_SPECS[name] = spec
    dve_ops._SUB_OPCODE_FOR_NAME[name] = row
    _CUSTOM["op"] = op
    return op


def _build_nc(mode="full", S=S_DEFAULT):
    from contextlib import ExitStack

    import concourse.mybir as mybir
    import concourse.tile as tile
    from concourse import bacc

    f32 = mybir.dt.float32
    bf16 = mybir.dt.bfloat16
    i16 = mybir.dt.int16
    RELU = mybir.ActivationFunctionType.Relu
    IDENT = mybir.ActivationFunctionType.Identity
    ADD = mybir.AluOpType.add
    MAX = mybir.AluOpType.max
    MULT = mybir.AluOpType.mult

    relu_add = _ensure_custom_op()
    S = tuple(S)
    Q = CHI * sum(S)
    cbase = [CHI * sum(S[:c]) for c in range(NCH)]

    nc = bacc.Bacc("TRN2", target_bir_lowering=False, debug=False,
                   enable_asserts=False, num_devices=NC)

    def din(name, shape, dt=None):
        return nc.dram_tensor(name, shape, dt or f32, kind="ExternalInput").ap()

    peTA_d = din("peTA_sh", [34, Q], bf16)   # edges^T ++ ones ++ padmask rows
    xgT_d = din("xgT_sh", [D, Q], bf16)      # host-gathered nodes^T (pads 0)
    idx_d = din("idx_sh", [128, 2 * (Q // 16)], i16)
    xT_d = din("xT", [D, N + SH], bf16)   # nodes.T ++ own-sorted.T
    Wb_d = din("Wb", [D, 5 * D], bf16)   # WnodeT0|WnT0|WnodeT1|WnT1|I
    WeA_d = din("WeA", [34, 2 * D], bf16)    # [WeT0;bne0;ones] | [WeT1;--]
    bias_d = din("bias", [D, 5])             # bne0|bn0|bne1|bn1|opse
    out_d = nc.dram_tensor("out", [SH, D], f32, kind="ExternalOutput").ap()

    with tile.TileContext(nc) as tc, ExitStack() as ctx:
        P = ctx.enter_context(tc.tile_pool(name="persist", bufs=1))
        dramp = ctx.enter_context(tc.tile_pool(name="dram", bufs=1, space="DRAM"))
        psumH = ctx.enter_context(tc.tile_pool(name="psumH", bufs=1, space="PSUM"))
        psumC = ctx.enter_context(tc.tile_pool(name="psumC", bufs=2, space="PSUM"))
        psumF = ctx.enter_context(tc.tile_pool(name="psumF", bufs=2, space="PSUM"))
        scrp = ctx.enter_context(tc.tile_pool(name="scr", bufs=3))

        # ---------------- inputs (2 queues, priority order) -------------
        # DMA_ENGINES serialize transfers in the cost model, so order and
        # granularity matter: critical small tensors first, big packed
        # tensors in chunk-pair pieces so compute starts early.
        xTA = P.tile([D, N + SH], bf16)
        nc.sync.dma_start(out=xTA[:], in_=xT_d[:])
        Wb = P.tile([D, 5 * D], bf16)
        nc.scalar.dma_start(out=Wb[:], in_=Wb_d[:])
        bias = P.tile([D, 5], f32)
        nc.scalar.dma_start(out=bias[:], in_=bias_d[:])
        WeA = P.tile([34, 2 * D], bf16)
        nc.scalar.dma_start(out=WeA[:], in_=WeA_d[:])
        peTA = P.tile([34, Q], bf16)
        xgT = P.tile([D, Q], bf16)
        idx12 = P.tile([128, 2 * (Q // 16)], i16)
        if IDX_EARLY:
            nc.sync.dma_start(out=idx12[:], in_=idx_d[:])
        bounds = [0] + [cbase[c] for c in range(2, NCH, 2)] + [Q]
        for i in range(len(bounds) - 1):
            lo, hi = bounds[i], bounds[i + 1]
            nc.sync.dma_start(out=peTA[:, lo:hi], in_=peTA_d[:, lo:hi])
            nc.scalar.dma_start(out=xgT[:, lo:hi], in_=xgT_d[:, lo:hi])
            if i == 0 and not IDX_EARLY:
                nc.sync.dma_start(out=idx12[:], in_=idx_d[:])


        ident = Wb[:, 4 * D:5 * D]
        hbT = P.tile([D, HBW], f32)
        nc.gpsimd.memset(hbT[:, N:HBW], -1e9)
        dve_scrap = P.tile([128, 1], f32)

        def fold(r, c, Sc, msg, tree_eng):
            """r [128, CHI, Sc] bf16 -> msg[:, chunk c] via L1 tree + reduce."""
            teng = nc.gpsimd if tree_eng == "P" else nc.vector
            h1 = Sc // 2
            t1 = scrp.tile([128, CHI, h1], bf16, tag=f"t1{Sc}")
            teng.tensor_tensor(out=t1[:], in0=r[:, :, 0:h1],
                               in1=r[:, :, h1:Sc], op=ADD)
            if h1 % 2 == 0:
                h2 = h1 // 2
                t2 = scrp.tile([128, CHI, h2], bf16, tag=f"t2{Sc}")
                teng.tensor_tensor(out=t2[:], in0=t1[:, :, 0:h2],
                                   in1=t1[:, :, h2:h1], op=ADD)
            else:
                t2 = t1
            nc.vector.tensor_reduce(
                out=msg[:, CHI * c:CHI * (c + 1)], in_=t2[:],
                axis=mybir.AxisListType.X, op=ADD)

        def finish(l, msg, xsT_l):
            WnT = Wb[:, 2 * D * l + D:2 * D * l + 2 * D]
            bn = bias[:, 2 * l + 1:2 * l + 2]
            hT = P.tile([D, SH], bf16, tag=f"hT{l}")
            for hh in range(2):
                sl = slice(64 * hh, 64 * (hh + 1))
                xt, xo = xsT_l
                z_bf = P.tile([D, 64], bf16, tag=f"zbf{l}{hh}")
                nc.vector.scalar_tensor_tensor(
                    out=z_bf[:], in0=xt[:, xo + 64 * hh:xo + 64 * (hh + 1)],
                    scalar=bias[:, 4:5], in1=msg[:, sl], op0=MULT, op1=ADD)
                ps_h = psumF.tile([D, 64], f32, tag="fin")
                nc.tensor.matmul(out=ps_h[:], lhsT=WnT, rhs=z_bf[:],
                                 start=True, stop=True)
                nc.scalar.activation(out=hT[:, sl], in_=ps_h[:], func=RELU,
                                     bias=bn)
            return hT

        # ---------------- layer 0: host-pregathered node term ----------
        def layer0(xsT_l):
            WnodeT = Wb[:, 0:D]
            msg = P.tile([D, SH], f32, tag="msg0")
            for c in range(NCH):
                W = CHI * S[c]
                ps = psumC.tile([128, W], f32, tag="chunk")
                for s0 in range(0, W, 512):
                    s1 = min(s0 + 512, W)
                    nc.tensor.matmul(out=ps[:, s0:s1], lhsT=WeA[:, 0:D],
                                     rhs=peTA[:, cbase[c] + s0:cbase[c] + s1],
                                     start=True, stop=False)
                    nc.tensor.matmul(out=ps[:, s0:s1], lhsT=WnodeT,
                                     rhs=xgT[:, cbase[c] + s0:cbase[c] + s1],
                                     start=False, stop=True)
                r = scrp.tile([128, CHI, S[c]], bf16, tag=f"r{S[c]}")
                r2 = r[:].rearrange("p a b -> p (a b)")
                if L0_EXIT[c] == "A":
                    nc.scalar.activation(out=r2, in_=ps[:], func=RELU)
                else:
                    nc.vector.tensor_scalar(out=r2, in0=ps[:], scalar1=0.0,
                                            scalar2=None, op0=MAX)
                fold(r, c, S[c], msg, TREE0[c])
            return finish(0, msg, xsT_l)

        # ------------- layer 1: on-device hb + POOL gather --------------
        def hb_prep(l, xT_l, idx_half):
            WnodeT = Wb[:, 2 * D * l:2 * D * l + D]
            bne = bias[:, 2 * l:2 * l + 1]
            iof = idx_half * (Q // 16)
            psH = psumH.tile([D, N], f32, tag="hb")
            for h in range(2):
                nc.tensor.matmul(out=psH[:, 512 * h:512 * (h + 1)],
                                 lhsT=WnodeT, rhs=xT_l(h),
                                 start=True, stop=True)
            if HB_SPLIT:
                nc.vector.tensor_scalar(out=hbT[:, 0:512], in0=psH[:, 0:512],
                                        scalar1=bne, scalar2=None, op0=ADD)
                nc.scalar.activation(out=hbT[:, 512:N], in_=psH[:, 512:N],
                                     func=IDENT, bias=bne)
            elif HB_ENG == "D":
                nc.vector.tensor_scalar(out=hbT[:, 0:N], in0=psH[:],
                                        scalar1=bne, scalar2=None, op0=ADD)
            else:
                nc.scalar.activation(out=hbT[:, 0:N], in_=psH[:],
                                     func=IDENT, bias=bne)
            hbg = P.tile([D, Q], f32, tag=f"hbg{l}")
            per = (NCH + GATHER_SPLIT - 1) // GATHER_SPLIT
            for g in range(0, NCH, per):
                lo = cbase[g]
                hi = cbase[g + per] if g + per < NCH else Q
                nc.gpsimd.ap_gather(
                    out_ap=hbg[:, lo:hi], in_ap=hbT[:],
                    idxs_ap=idx12[:, iof + lo // 16:iof + hi // 16],
                    channels=128, num_elems=HBW, d=1, num_idxs=hi - lo)
            hgbs = {}

            def conv(c):
                W = CHI * S[c]
                hgb = scrp.tile([128, W], bf16, tag=f"hgb{S[c]}")
                if L1_EXIT[c] == "P":
                    nc.gpsimd.tensor_scalar(
                        out=hgb[:], in0=hbg[:, cbase[c]:cbase[c] + W],
                        scalar1=0.0, scalar2=None, op0=ADD)
                else:
                    nc.scalar.activation(
                        out=hgb[:], in_=hbg[:, cbase[c]:cbase[c] + W],
                        func=IDENT)
                hgbs[c] = hgb

            if CONV_PREP:
                for c in range(NCH):
                    if L1_EXIT[c] in "AP":
                        conv(c)
            return hbg, hgbs, conv

        def layer1(l, hbg, hgbs, conv, xsT_l):
            WeT = WeA[0:32, D:2 * D]
            msg = P.tile([D, SH], f32, tag=f"msg{l}")
            for c in range(NCH):
                W = CHI * S[c]
                act = L1_EXIT[c] in "AP"
                ps = psumC.tile([128, W], f32, tag="chunk")
                if act:
                    if c not in hgbs:
                        conv(c)
                    hgb = hgbs[c]
                for s0 in range(0, W, 512):
                    s1 = min(s0 + 512, W)
                    nc.tensor.matmul(out=ps[:, s0:s1], lhsT=WeT,
                                     rhs=peTA[0:32, cbase[c] + s0:cbase[c] + s1],
                                     start=True, stop=not act)
                    if act:
                        nc.tensor.matmul(out=ps[:, s0:s1], lhsT=ident,
                                         rhs=hgb[:, s0:s1],
                                         start=False, stop=True)
                r = scrp.tile([128, CHI, S[c]], bf16, tag=f"r{S[c]}")
                r2 = r[:].rearrange("p a b -> p (a b)")
                if act:
                    nc.scalar.activation(out=r2, in_=ps[:], func=RELU)
                else:
                    nc.vector._custom_dve(
                        relu_add, out=r2, in0=ps[:],
                        in1=hbg[:, cbase[c]:cbase[c] + W],
                        accum_out=dve_scrap[:])
                fold(r, c, S[c], msg, TREE1[c])
            return finish(1, msg, xsT_l)

        def x0(h):
            return xTA[:, 512 * h:512 * (h + 1)]

        # ---------------- wiring ----------------
        if mode == "l1":
            h2T = layer0((xTA, N))
        elif mode == "nocc":
            hbg1, hgbs1, conv1 = hb_prep(1, x0, 0)  # hoisted
            h1T = layer0((xTA, N))
            h2T = layer1(1, hbg1, hgbs1, conv1, (h1T, 0))
        elif mode == "full":
            h1T = layer0((xTA, N))
            # ------------- allgather updated node features -------------
            ps_t = psumF.tile([SH, D], bf16, tag="fin")
            nc.tensor.transpose(ps_t[:], h1T[:], ident)
            h1_own = P.tile([SH, D], f32)
            nc.scalar.copy(h1_own[:], ps_t[:])

            gin = dramp.tile([SH, D], f32)
            gout = dramp.tile([N, D], f32)
            nc.gpsimd.dma_start(out=gin[:], in_=h1_own[:])
            nc.gpsimd.collective_compute(
                "AllGather", mybir.AluOpType.bypass,
                replica_groups=[list(range(NC))],
                ins=[gin[:].opt()], outs=[gout[:].opt()])

            x1b = P.tile([128, N // 128, D], bf16)
            nc.gpsimd.dma_start(
                out=x1b[:], in_=gout[:].rearrange("(jt p) d -> p jt d", p=128))
            x1T = P.tile([D, N // 128, 128], bf16)
            nc.sync.dma_start(out=x1T[:], in_=x1b[:], transpose=True)

            def x1(h):
                return x1T[:, 4 * h:4 * (h + 1), :]

            hbg1, hgbs1, conv1 = hb_prep(1, x1, 1)
            h2T = layer1(1, hbg1, hgbs1, conv1, (h1T, 0))
        else:
            raise ValueError(mode)

        # ---------------- output (rows in sorted order) ----------------
        h2_own = P.tile([SH, D], f32)
        for hh in range(2):
            sl = slice(64 * hh, 64 * (hh + 1))
            ps_o = psumF.tile([64, D], bf16, tag="fin")
            nc.tensor.transpose(ps_o[:], h2T[:, sl], ident)
            nc.scalar.copy(h2_own[sl, :], ps_o[:])
            nc.sync.dma_start(out=out_d[sl, :], in_=h2_own[sl, :])

    nc.compile()
    return nc


def _plan(adj):
    """Degree-sort rows per core, bucket into NCH chunks, pad to mult of 4."""
    deg = adj.sum(1).astype(np.int64).reshape(NC, SH)
    perms = [np.argsort(-deg[c], kind="stable") for c in range(NC)]
    S = []
    for ch in range(NCH):
        mx = max(int(deg[c][perms[c][CHI * ch:CHI * (ch + 1)]].max())
                 for c in range(NC))
        S.append(max(4, int(-(-mx // 4) * 4)))
    return perms, tuple(S)


def _wrap_idx(L):
    """ap_gather index layout: [128, Q//16], idx[p, m] = L[m*16 + p%16]."""
    w = L.reshape(-1, 16).T.astype(np.int16)          # [16, Q//16]
    return np.tile(w, (8, 1))


def _host_inputs(inputs):
    """Build the 8 per-core input maps + plan from full inputs."""
    import ml_dtypes

    bf = ml_dtypes.bfloat16
    adj = np.asarray(inputs["adj"], np.float32)
    nodes = np.asarray(inputs["nodes"], np.float32)
    edges = np.asarray(inputs["edges"], np.float32)
    eps = float(np.asarray(inputs["eps"], np.float32).reshape(-1)[0])
    perms, S = _plan(adj)
    Q = CHI * sum(S)
    cbase = [CHI * sum(S[:c]) for c in range(NCH)]

    # global position of node j in the allgathered (per-core sorted) layout
    gpos = np.empty(N, np.int64)
    for c in range(NC):
        gpos[c * SH + perms[c]] = c * SH + np.arange(SH)

    Wne = [np.asarray(inputs["Wne0"], np.float32),
           np.asarray(inputs["Wne1"], np.float32)]
    Wb = np.concatenate(
        [np.concatenate(
            [Wne[l][:, :D].T,
             np.asarray(inputs[f"Wn{l}"], np.float32).T], axis=1)
         for l in range(2)] + [np.eye(D, dtype=np.float32)], axis=1)
    # WeA: [WeT_l ; bne_l ; ones] stacked per layer -> [34, 2D]
    WeA = np.zeros((34, 2 * D), np.float32)
    for l in range(2):
        WeA[0:32, D * l:D * (l + 1)] = Wne[l][:, D:D + E].T
        WeA[32, D * l:D * (l + 1)] = np.asarray(inputs[f"bne{l}"], np.float32)
        WeA[33, D * l:D * (l + 1)] = 1.0
    bias = np.stack(
        [np.asarray(inputs["bne0"], np.float32),
         np.asarray(inputs["bn0"], np.float32),
         np.asarray(inputs["bne1"], np.float32),
         np.asarray(inputs["bn1"], np.float32),
         np.full(D, 1.0 + eps, np.float32)], axis=1)
    com = {
        "Wb": np.ascontiguousarray(Wb.astype(bf)),
        "WeA": np.ascontiguousarray(WeA.astype(bf)),
        "bias": np.ascontiguousarray(bias),
    }

    maps = []
    for c in range(NC):
        perm = perms[c]
        rows = c * SH + perm                       # global ids, sorted order
        pea = np.zeros((Q, 34), np.float32)
        pea[:, 33] = -1e9                          # pad mask row
        L1 = np.full(Q, PAD, np.int64)
        L2 = np.full(Q, PAD, np.int64)
        xg = np.zeros((Q, D), np.float32)
        for p in range(SH):
            ch = p // CHI
            il = p % CHI
            base = cbase[ch] + il * S[ch]
            nbr = np.nonzero(adj[rows[p]])[0]
            k = len(nbr)
            assert k <= S[ch]
            pea[base:base + k, 0:E] = edges[rows[p], nbr]
            pea[base:base + k, 32] = 1.0           # bias carrier
            pea[base:base + k, 33] = 0.0           # not padded
            xg[base:base + k] = nodes[nbr]
            L1[base:base + k] = nbr
            L2[base:base + k] = gpos[nbr]
        m = dict(com)
        m["xT"] = np.ascontiguousarray(
            np.concatenate([nodes.T, nodes[rows].T], axis=1).astype(bf))
        m["peTA_sh"] = np.ascontiguousarray(pea.T.astype(bf))
        m["xgT_sh"] = np.ascontiguousarray(xg.T.astype(bf))
        m["idx_sh"] = np.ascontiguousarray(
            np.concatenate([_wrap_idx(L1), _wrap_idx(L2)], axis=1))
        maps.append(m)
    return maps, perms, S


def _get_runner(S):
    """Build (once per S) a cached jit(shard_map) callable."""
    key = ("runner", S)
    if key in _cache:
        return _cache[key]
    import jax
    from jax.sharding import Mesh, PartitionSpec, NamedSharding
    from jax.experimental.shard_map import shard_map
    import concourse.mybir as mybir
    from concourse import bass2jax
    from concourse.bass2jax import _bass_exec_p, partition_id_tensor

    nckey = ("nc", S)
    if nckey not in _cache:
        _cache[nckey] = _build_nc("full", S)
    nc = _cache[nckey]
    bass2jax.install_neuronx_cc_hook()

    in_names, out_names, out_avals, zero_outs = [], [], [], []
    partition_name = nc.partition_id_tensor.name if nc.partition_id_tensor else None
    for alloc in nc.m.functions[0].allocations:
        if not isinstance(alloc, mybir.MemoryLocationSet):
            continue
        name = alloc.memorylocations[0].name
        if alloc.kind == "ExternalInput":
            if name != partition_name:
                in_names.append(name)
        elif alloc.kind == "ExternalOutput":
            shape = list(alloc.tensor_shape)
            dtype = np.dtype(mybir.dt.np(alloc.dtype))
            out_avals.append(jax.core.ShapedArray(shape, dtype))
            out_names.append(name)
            zero_outs.append(np.zeros(shape, dtype))

    n_params = len(in_names)
    all_in_names = list(in_names) + list(out_names)
    if partition_name is not None:
        all_in_names.append(partition_name)

    def _body(*args):
        operands = list(args)
        if partition_name is not None:
            operands.append(partition_id_tensor())
        outs = _bass_exec_p.bind(
            *operands,
            out_avals=tuple(out_avals),
            in_names=tuple(all_in_names),
            out_names=tuple(out_names),
            lowering_input_output_aliases=(),
            sim_require_finite=True,
            sim_require_nnan=True,
            nc=nc,
        )
        return tuple(outs)

    devices = jax.devices()[:NC]
    mesh = Mesh(np.asarray(devices), ("core",))
    n_outs = len(out_names)
    fn = jax.jit(
        shard_map(_body, mesh=mesh,
                  in_specs=(PartitionSpec("core"),) * (n_params + n_outs),
                  out_specs=(PartitionSpec("core"),) * n_outs,
                  check_rep=False),
        keep_unused=True)
    sh = NamedSharding(mesh, PartitionSpec("core"))
    dev_zeros = [
        jax.device_put(np.zeros((NC * z.shape[0], *z.shape[1:]), z.dtype), sh)
        for z in zero_outs
    ]

    def run(maps):
        dev_in = []
        for nm in in_names:
            arrs = [
                jax.device_put(np.asarray(maps[c][nm]), devices[c])
                for c in range(NC)
            ]
            shp = arrs[0].shape
            glob = jax.make_array_from_single_device_arrays(
                (NC * shp[0], *shp[1:]), sh, arrs)
            dev_in.append(glob)
        outs = fn(*dev_in, *dev_zeros)
        oi = out_names.index("out")
        return np.asarray(outs[oi]).reshape(NC, SH, D)

    _cache[key] = run
    return run


def kernel(**inputs):
    maps, perms, S = _host_inputs(inputs)
    run = _get_runner(S)
    raw = run(maps)                                # [NC, SH, D], sorted rows
    out = np.empty((N, D), np.float32)
    for c in range(NC):
        out[c * SH + perms[c]] = raw[c]
    return np.ascontiguousarray(out.astype(np.float32))


if __name__ == "__main__":
    _build_nc("nocc")
    print("build+compile OK")
